# revision 2
# baseline (speedup 1.0000x reference)
"""ChildSum TreeLSTM on 8 Trainium2 NeuronCores (Bass/Tile) — v2.

Latency-first redesign, everything SBUF-resident, f16 data path:
  - Host: node levels; bottom (level < L0) split into complete subtrees
    bin-packed over 8 cores; top region (level >= L0, ~129 nodes) replicated
    on all cores after one f16 AllReduce of cut-edge contributions.
  - Device: transposed pipeline [feature-chunk partitions x node columns].
    Per piece: JIT phase A (x-side preacts matmul'd straight into PSUM,
    biases folded via a ones row), child-sum segment reduction as direct
    PE pair-matmuls against SBUF-resident per-piece contribution tiles
    (no DRAM round trip, no indirect DMA), activations read PSUM,
    h and f*c transposed back by PE and cast-copied into f16 contrib tiles.
"""

import numpy as np
from contextlib import ExitStack

N_CORES = 8
P = 128
IN_DIM = 300
MD = 256
TOP_CAP = 160
NKT = 3          # k-tiles over IN_DIM+1 (incl. ones/bias row)
NCH = 8          # feature chunks: 0-3 = i,o | 4-5 = u | 6-7 = f
TOPS = 132       # padded top phase-A columns (128 tile0 + root + pad)
CCW = 132        # cut-contribution (AllReduce) columns: 128 tile0 + root + pad
ONES_ROW = IN_DIM - 2 * P  # local row 44 of k-tile 2
K2 = ONES_ROW + 1          # used rows of k-tile 2 (44 emb rows + ones)

F16 = np.float16  # device data dtype


# ---------------------------------------------------------------- host side


def _pieces(lb, nl):
    out = []
    s = lb
    end = lb + nl
    while s < end:
        take = min(P, end - s)
        out.append((s, take))
        s += take
    return out


def _preprocess(parent):
    parent = np.asarray(parent, dtype=np.int64)
    N = len(parent)
    level = np.zeros(N, dtype=np.int64)
    for j in range(N - 1):
        p = parent[j]
        if level[p] <= level[j]:
            level[p] = level[j] + 1
    n_levels = int(level.max()) + 1

    cnt_ge = np.zeros(n_levels + 1, dtype=np.int64)
    for l in range(n_levels - 1, -1, -1):
        cnt_ge[l] = cnt_ge[l + 1] + int((level == l).sum())
    L0 = n_levels
    for l in range(n_levels + 1):
        if cnt_ge[l] <= TOP_CAP:
            L0 = l
            break
    assert 1 <= L0 < n_levels, f"degenerate tree: L0={L0} n_levels={n_levels}"
    assert cnt_ge[L0] <= 129, f"top too large: {cnt_ge[L0]}"

    is_top = level >= L0
    root = N - 1
    # top slots: tile0 slots 0.. for non-root top nodes (level-major); root=128
    tslot = np.full(N, -1, dtype=np.int64)
    TLB = []
    b = 0
    for l in range(L0, n_levels):
        sel = np.arange(N)[(level == l) & is_top & (np.arange(N) != root)]
        TLB.append((b, len(sel)))
        tslot[sel] = b + np.arange(len(sel))
        b += len(sel)
    assert b <= P, f"non-root top nodes {b} > 128"
    tslot[root] = P

    # subtree partition of the bottom across cores
    anchor = np.full(N, -1, dtype=np.int64)
    for j in range(N - 1, -1, -1):
        if is_top[j]:
            continue
        p = parent[j]
        anchor[j] = j if (p == N or is_top[p]) else anchor[p]
    roots = np.unique(anchor[anchor >= 0])
    sizes = np.zeros(N, dtype=np.int64)
    np.add.at(sizes, anchor[anchor >= 0], 1)
    order = roots[np.argsort(-sizes[roots], kind="stable")]
    load = np.zeros(N_CORES, dtype=np.int64)
    core_of_root = {}
    for r in order:
        c = int(np.argmin(load))
        core_of_root[int(r)] = c
        load[c] += sizes[r]
    core = np.full(N, -1, dtype=np.int64)
    bot = anchor >= 0
    core[bot] = [core_of_root[int(a)] for a in anchor[bot]]

    nodes_cl = [[[] for _ in range(L0)] for _ in range(N_CORES)]
    for j in np.arange(N)[bot]:
        nodes_cl[int(core[j])][int(level[j])].append(int(j))
    NL = [max(len(nodes_cl[c][l]) for c in range(N_CORES)) for l in range(L0)]

    LB = []
    b = 0
    for l in range(L0):
        LB.append(b)
        b += NL[l]
    NBOT = b
    NPHA = NBOT + TOPS

    # Assign slots top-down (level L0-1 .. 0) ordering each level's nodes by
    # their parent's already-assigned position; children of any target piece
    # then occupy contiguous slot ranges -> far fewer, denser pair matrices.
    slot_of = np.full(N, -1, dtype=np.int64)
    node_at = [np.full(NPHA, -1, dtype=np.int64) for _ in range(N_CORES)]
    for j in np.arange(N)[is_top]:
        for c in range(N_CORES):
            node_at[c][NBOT + tslot[j]] = j
    for l in range(L0 - 1, -1, -1):
        for c in range(N_CORES):
            def pkey(j):
                p = int(parent[j])
                if p == N:
                    return (2, 0)
                if is_top[p]:
                    return (1, int(tslot[p]))
                return (0, int(slot_of[p]))
            for i, j in enumerate(sorted(nodes_cl[c][l], key=pkey)):
                slot_of[j] = LB[l] + i
                node_at[c][LB[l] + i] = j

    # ---- global piece list ----
    pieces = []
    for l in range(L0):
        for (s0, cnt) in _pieces(LB[l], NL[l]):
            pieces.append(dict(kind="bot", level=l, s0=s0, cnt=cnt))
    NBP = len(pieces)
    for li, l in enumerate(range(L0, n_levels)):
        base, cnt = TLB[li]
        if cnt:
            pieces.append(dict(kind="top", level=l, base=base, cnt=cnt))
    pieces.append(dict(kind="root", level=n_levels - 1, base=P, cnt=1))

    pidx_of_slot = np.full(NBOT, -1, dtype=np.int64)
    for i, pc in enumerate(pieces[:NBP]):
        pidx_of_slot[pc["s0"]:pc["s0"] + pc["cnt"]] = i

    child_by_parent = [[] for _ in range(N)]
    for j in range(N - 1):
        child_by_parent[int(parent[j])].append(j)

    # ---- per-core pair S matrices + cross-core union layout ----
    # pairsS[c][t] : {src_piece: S[P, P]}  (cols padded to P)
    pairsS = [[{} for _ in range(NBP)] for _ in range(N_CORES)]
    for c in range(N_CORES):
        for t, pc in enumerate(pieces[:NBP]):
            if pc["level"] == 0:
                continue
            for pl in range(pc["cnt"]):
                node = node_at[c][pc["s0"] + pl]
                if node < 0:
                    continue
                for ch in child_by_parent[int(node)]:
                    cs = int(slot_of[ch])
                    sp = int(pidx_of_slot[cs])
                    r = cs - pieces[sp]["s0"]
                    S = pairsS[c][t].setdefault(
                        sp, np.zeros((P, P), np.float32))
                    S[r, pl] = 1.0
    pairs_union = [sorted(set().union(*[pairsS[c][t].keys()
                                        for c in range(N_CORES)]))
                   for t in range(NBP)]
    pair_off = {}
    off = 0
    for t in range(NBP):
        w = pieces[t]["cnt"]
        for sp in pairs_union[t]:
            pair_off[(t, sp)] = off
            off += w
    NPAIRC = max(off, P)
    # staged-load split points (end of pairs into levels <=1 / <=3)
    spA = spB = off
    for t in range(NBP):
        if pieces[t]["level"] == 2 and pairs_union[t]:
            spA = min(spA, pair_off[(t, pairs_union[t][0])])
        if pieces[t]["level"] == 4 and pairs_union[t]:
            spB = min(spB, pair_off[(t, pairs_union[t][0])])

    # ---- cut-edge S (bottom piece -> ccT columns = tslot), union ----
    cutS = [[None] * NBP for _ in range(N_CORES)]
    for c in range(N_CORES):
        for j in range(N - 1):
            p = int(parent[j])
            if is_top[j] or core[j] != c or p == N or not is_top[p]:
                continue
            cs = int(slot_of[j])
            sp = int(pidx_of_slot[cs])
            r = cs - pieces[sp]["s0"]
            if cutS[c][sp] is None:
                cutS[c][sp] = np.zeros((P, CCW), np.float32)
            cutS[c][sp][r, int(tslot[p])] = 1.0
    cut_union = sorted({t for c in range(N_CORES) for t in range(NBP)
                        if cutS[c][t] is not None})
    cut_off = {t: i * CCW for i, t in enumerate(cut_union)}
    NCUTC = max(len(cut_union) * CCW, CCW)

    # ---- top S (same on all cores) ----
    topS = []
    for pc in pieces[NBP:]:
        cnt = pc["cnt"]
        S = np.zeros((P, P), np.float32)
        if pc["kind"] == "root":
            nodes = [root]
        else:
            nodes = [node_at[0][NBOT + pc["base"] + i] for i in range(cnt)]
        for pl, node in enumerate(nodes):
            for ch in child_by_parent[int(node)]:
                if is_top[ch] and int(ch) != root:
                    S[int(tslot[ch]), pl] = 1.0
        topS.append(S)
    NTOPC = len(topS) * P

    meta = dict(
        N=N, L0=L0, n_levels=n_levels, level=level, parent=parent,
        NL=NL, LB=LB, NBOT=NBOT, NPHA=NPHA, NBP=NBP,
        tslot=tslot, TLB=TLB, slot_of=slot_of, node_at=node_at, core=core,
        is_top=is_top, pieces=pieces,
        pairsS=pairsS, pairs_union=pairs_union, pair_off=pair_off,
        NPAIRC=NPAIRC, spA=spA, spB=spB,
        cutS=cutS, cut_union=cut_union, cut_off=cut_off,
        NCUTC=NCUTC, topS=topS, NTOPC=NTOPC,
    )
    return meta


def _to_f16(x):
    return np.ascontiguousarray(np.asarray(x, dtype=np.float32).astype(F16))


def _host_arrays(meta, embs, Wx, bx, Wh, bh, Wfh, bfh):
    """Float32 host-side input arrays (pre-f16), per core."""
    N = meta["N"]
    NPHA = meta["NPHA"]
    NBP = meta["NBP"]
    parent = meta["parent"]

    wxk = np.zeros((P, NKT, 1024), np.float32)
    for t in range(NKT):
        k0 = t * P
        kn = min(P, IN_DIM - k0)
        wxk[:kn, t, :] = Wx[k0:k0 + kn, :]
    bias = np.concatenate([bx[:768] + bh, bx[768:] + bfh])
    wxk[ONES_ROW, 2, :] = bias

    whk = np.stack([Wh[:P, :], Wh[P:, :]], axis=1)      # [P, 2, 768]
    wfhk = np.stack([Wfh[:P, :], Wfh[P:, :]], axis=1)   # [P, 2, 256]

    embs_pad = np.concatenate([embs, np.zeros((1, IN_DIM), np.float32)], 0)
    emb_k = np.zeros((P, NKT, N + 1), np.float32)
    for t in range(NKT):
        k0 = t * P
        kn = min(P, IN_DIM - k0)
        emb_k[:kn, t, :] = embs_pad[:, k0:k0 + kn].T
    emb_k[ONES_ROW, 2, :N] = 1.0

    per_core = []
    for c in range(N_CORES):
        na = meta["node_at"][c]
        sel = np.where(na >= 0, na, N)
        par = np.where(na >= 0, parent[np.clip(na, 0, N - 1)], N)
        par = np.minimum(par, N)
        embs4 = np.empty((P, 4, NPHA), np.float32)
        embs4[:, 0:2, :] = emb_k[:, 0:2, sel]
        embs4[:, 2:4, :] = emb_k[:, 0:2, par]
        embs2 = np.empty((K2, 2, NPHA), np.float32)
        embs2[:, 0, :] = emb_k[0:K2, 2, sel]
        embs2[:, 1, :] = emb_k[0:K2, 2, par]

        spair = np.zeros((P, meta["NPAIRC"]), np.float32)
        for t in range(NBP):
            w = meta["pieces"][t]["cnt"]
            for sp in meta["pairs_union"][t]:
                S = meta["pairsS"][c][t].get(sp)
                if S is not None:
                    o = meta["pair_off"][(t, sp)]
                    spair[:, o:o + w] = S[:, :w]

        scut = np.zeros((P, meta["NCUTC"]), np.float32)
        for t in meta["cut_union"]:
            S = meta["cutS"][c][t]
            if S is not None:
                o = meta["cut_off"][t]
                scut[:, o:o + CCW] = S

        stop = np.concatenate(meta["topS"], axis=1)

        per_core.append(dict(embs4=embs4, embs2=embs2, wxk=wxk.reshape(P, -1),
                             whk=whk.reshape(P, -1), wfhk=wfhk.reshape(P, -1),
                             ident=np.eye(P, dtype=np.float32),
                             spair=spair, scut=scut, stop=stop))
    return per_core


def _build_inputs(meta, embs, Wx, bx, Wh, bh, Wfh, bfh):
    return [{k: _to_f16(v) for k, v in m.items()}
            for m in _host_arrays(meta, embs, Wx, bx, Wh, bh, Wfh, bfh)]


# ------------------------------------------------------------- numpy emulator


def _emulate(meta, per_core, rnd=None):
    """Numpy re-implementation of the device program (fp32). Returns h[N,256].

    rnd: optional fn rounding SBUF-resident intermediates (e.g. to fp16)."""
    N = meta["N"]
    NBOT = meta["NBOT"]
    NBP = meta["NBP"]
    pieces = meta["pieces"]
    if rnd is None:
        def rnd(x):
            return x

    def sig(x):
        return 1.0 / (1.0 + np.exp(-x))

    outs = []
    ccT_sum = np.zeros((P, 4, CCW), np.float32)
    percore_state = []
    for c in range(N_CORES):
        d = per_core[c]
        embs4 = d["embs4"]
        embs2 = d["embs2"]
        wxk = d["wxk"].reshape(P, NKT, 1024)
        whk = d["whk"].reshape(P, 2, 768)
        wfhk = d["wfhk"].reshape(P, 2, 256)
        contrib = [np.zeros((P, 512), np.float32) for _ in range(NBP)]
        G = [None] * NBP
        ccT = np.zeros((P, 4, CCW), np.float32)
        outT = np.zeros((P, 2, meta["NPHA"]), np.float32)

        for T, pc in enumerate(pieces[:NBP]):
            s0, cnt = pc["s0"], pc["cnt"]
            # phase A
            pre = np.zeros((P, NCH, cnt), np.float32)
            for ch in range(NCH):
                side = 0 if ch < 6 else 2
                w2 = 0 if ch < 6 else 1
                for t in range(2):
                    pre[:, ch, :] += (wxk[:, t, ch * P:(ch + 1) * P].T
                                      @ embs4[:, side + t, s0:s0 + cnt])
                pre[:, ch, :] += (wxk[0:K2, 2, ch * P:(ch + 1) * P].T
                                  @ embs2[:, w2, s0:s0 + cnt])
            # seg via G association
            if pc["level"] > 0:
                segfc = np.zeros((P, 2, cnt), np.float32)
                iouT = np.zeros((6 * P, cnt), np.float32)
                for sp in meta["pairs_union"][T]:
                    S = meta["pairsS"][c][T].get(sp)
                    if S is None:
                        continue
                    iouT += G[sp].T @ S[:, :cnt]
                    for ch in range(2):
                        segfc[:, ch, :] += (
                            contrib[sp][:, 256 + ch * P:256 + (ch + 1) * P].T
                            @ S[:, :cnt])
                pre[:, 0:6, :] += iouT.reshape(6, P, cnt).transpose(1, 0, 2)
            io = rnd(sig(pre[:, 0:4, :]))
            u = rnd(np.tanh(pre[:, 4:6, :]))
            c_ = rnd(io[:, 0:2, :] * u)
            if pc["level"] > 0:
                c_ = rnd(c_ + rnd(segfc))
            tc = rnd(np.tanh(c_))
            hT = rnd(io[:, 2:4, :] * tc)
            outT[:, :, s0:s0 + cnt] = hT
            fpre = pre[:, 6:8, :].copy()
            for ch in range(2):
                for t in range(2):
                    fpre[:, ch, :] += (wfhk[:, t, ch * P:(ch + 1) * P].T
                                       @ hT[:, t, :])
            f = rnd(sig(fpre))
            fcT = rnd(f * c_)
            hT2 = np.concatenate([hT[:, 0], hT[:, 1]], 0)   # [256, cnt]
            Gt = np.zeros((P, 768), np.float32)
            Gt[:cnt, :] = rnd(hT2.T @ np.concatenate([whk[:, 0], whk[:, 1]], 0))
            con = np.concatenate([hT[:, 0], hT[:, 1], fcT[:, 0], fcT[:, 1]], 0)
            contrib[T][:cnt, :] = con.T  # [cnt, 512]
            G[T] = Gt
            if meta["cutS"][c][T] is not None:
                S = meta["cutS"][c][T]
                for ch in range(4):
                    ccT[:, ch, :] += contrib[T][:, ch * P:(ch + 1) * P].T @ S
        ccT_sum += ccT
        percore_state.append(dict(outT=outT, contrib=contrib, embs4=embs4,
                                  embs2=embs2, wxk=wxk, whk=whk, wfhk=wfhk))

    # top (replicated; emulate once with core 0 data)
    st = percore_state[0]
    topC0 = np.zeros((P, 256), np.float32)
    hT_top = np.zeros((P, 2, P), np.float32)
    fcT_top = np.zeros((P, 2, P), np.float32)
    G_top = None
    stopS = meta["topS"]
    for ti, pc in enumerate(pieces[NBP:]):
        base, cnt = pc["base"], pc["cnt"]
        col0 = NBOT + base
        pre = np.zeros((P, NCH, cnt), np.float32)
        for ch in range(NCH):
            side = 0 if ch < 6 else 2
            w2 = 0 if ch < 6 else 1
            for t in range(2):
                pre[:, ch, :] += (st["wxk"][:, t, ch * P:(ch + 1) * P].T
                                  @ st["embs4"][:, side + t, col0:col0 + cnt])
            pre[:, ch, :] += (st["wxk"][0:K2, 2, ch * P:(ch + 1) * P].T
                              @ st["embs2"][:, w2, col0:col0 + cnt])
        pre = rnd(pre)  # xt_top round-trips through fp16 SBUF
        S = stopS[ti]
        whf = np.concatenate([st["whk"][:, 0], st["whk"][:, 1]], 0)  # [256,768]
        iouT = np.zeros((6 * P, cnt), np.float32)
        if G_top is not None:
            iouT += G_top.T @ S[:, :cnt]
        ccr = rnd(ccT_sum)
        cch = np.concatenate([ccr[:, 0], ccr[:, 1]], 0)  # [256, CCW]
        iouT += whf[:, 0:768].T @ cch[:, base:base + cnt]
        pre[:, 0:6, :] += iouT.reshape(6, P, cnt).transpose(1, 0, 2)
        segfc = np.zeros((P, 2, cnt), np.float32)
        for ch in range(2):
            segfc[:, ch, :] += topC0[:, ch * P:(ch + 1) * P].T @ S[:, :cnt]
        segfc += ccr[:, 2:4, base:base + cnt]
        io = rnd(sig(pre[:, 0:4, :]))
        u = rnd(np.tanh(pre[:, 4:6, :]))
        c_ = rnd(rnd(io[:, 0:2, :] * u) + rnd(segfc))
        tc = rnd(np.tanh(c_))
        hT = rnd(io[:, 2:4, :] * tc)
        if pc["kind"] == "root":
            st["outT"][:, :, col0:col0 + cnt] = hT
            break
        hT_top[:, :, base:base + cnt] = hT
        hf = np.concatenate([hT_top[:, 0], hT_top[:, 1]], 0)  # [256, 128]
        G_top = rnd(hf.T @ whf)
        fpre = pre[:, 6:8, :].copy()
        for ch in range(2):
            for t in range(2):
                fpre[:, ch, :] += (st["wfhk"][:, t, ch * P:(ch + 1) * P].T
                                   @ hT[:, t, :])
        f = rnd(sig(fpre))
        fcT_top[:, :, base:base + cnt] = rnd(f * c_)
        topC0 = np.concatenate([fcT_top[:, 0], fcT_top[:, 1]], 0).T.copy()

    # assemble
    h = np.zeros((N, MD), np.float32)
    for c in range(N_CORES):
        na = meta["node_at"][c]
        oT = percore_state[c]["outT"]
        m = na[:NBOT] >= 0
        sl = np.arange(NBOT)[m]
        h[na[sl], 0:P] = oT[:, 0, sl].T
        h[na[sl], P:2 * P] = oT[:, 1, sl].T
    na0 = meta["node_at"][0]
    for ts in range(P):
        node = na0[NBOT + ts]
        if node >= 0:
            h[node, 0:P] = hT_top[:, 0, ts]
            h[node, P:2 * P] = hT_top[:, 1, ts]
    oT0 = percore_state[0]["outT"]
    h[N - 1, 0:P] = oT0[:, 0, NBOT + P]
    h[N - 1, P:2 * P] = oT0[:, 1, NBOT + P]
    return h


# ---------------------------------------------------------------- device side


def _build_program(meta, sim_no_collective=False):
    import concourse.tile as tile
    from concourse import bacc, mybir

    f32 = mybir.dt.float32
    f16 = mybir.dt.float16
    SIG = mybir.ActivationFunctionType.Sigmoid
    TANH = mybir.ActivationFunctionType.Tanh
    COPY = mybir.ActivationFunctionType.Copy

    NPHA = meta["NPHA"]
    NBOT = meta["NBOT"]
    NBP = meta["NBP"]
    pieces = meta["pieces"]
    NTOT = len(pieces)
    pairs_union = meta["pairs_union"]
    pair_off = meta["pair_off"]
    cut_union = meta["cut_union"]
    cut_off = meta["cut_off"]

    nc = bacc.Bacc("TRN2", target_bir_lowering=False, debug=False,
                   num_devices=N_CORES)

    embs4 = nc.dram_tensor("embs4", [P, 4, NPHA], f16, kind="ExternalInput").ap()
    embs2 = nc.dram_tensor("embs2", [K2, 2, NPHA], f16, kind="ExternalInput").ap()
    wxk = nc.dram_tensor("wxk", [P, NKT * 1024], f16, kind="ExternalInput").ap()
    whk = nc.dram_tensor("whk", [P, 2 * 768], f16, kind="ExternalInput").ap()
    wfhk = nc.dram_tensor("wfhk", [P, 2 * 256], f16, kind="ExternalInput").ap()
    identd = nc.dram_tensor("ident", [P, P], f16, kind="ExternalInput").ap()
    spaird = nc.dram_tensor("spair", [P, meta["NPAIRC"]], f16,
                            kind="ExternalInput").ap()
    scutd = nc.dram_tensor("scut", [P, meta["NCUTC"]], f16,
                           kind="ExternalInput").ap()
    stopd = nc.dram_tensor("stop", [P, meta["NTOPC"]], f16,
                           kind="ExternalInput").ap()

    outT_d = nc.dram_tensor("outT", [P, 2, NPHA], f16, kind="ExternalOutput").ap()
    topC_d = nc.dram_tensor("topC", [P, 512], f16, kind="ExternalOutput").ap()
    cc_in_h = nc.dram_tensor("cc_in_h", [P, 2 * CCW], f16).ap()
    cc_out_h = nc.dram_tensor("cc_out_h", [P, 2 * CCW], f16, addr_space="Shared").ap()
    cc_in_f = nc.dram_tensor("cc_in_f", [P, 2 * CCW], f16).ap()
    cc_out_f = nc.dram_tensor("cc_out_f", [P, 2 * CCW], f16, addr_space="Shared").ap()

    with tile.TileContext(nc) as tc, ExitStack() as ctx:
        wpool = ctx.enter_context(tc.tile_pool(name="weights", bufs=1))
        cpool = ctx.enter_context(tc.tile_pool(name="contrib", bufs=1))
        epool = ctx.enter_context(tc.tile_pool(name="embs", bufs=3))
        gpool = ctx.enter_context(tc.tile_pool(name="gates", bufs=2))
        pp_pre = ctx.enter_context(tc.tile_pool(name="ps_pre", bufs=2, space="PSUM"))
        pp_seg = ctx.enter_context(tc.tile_pool(name="ps_seg", bufs=2, space="PSUM"))
        pp_cc = ctx.enter_context(tc.tile_pool(name="ps_cc", bufs=1, space="PSUM"))

        wx_sb = wpool.tile([P, NKT, 1024], f16, name="wx_sb")
        wh_sb = wpool.tile([P, 2, 768], f16, name="wh_sb")
        wfh_sb = wpool.tile([P, 2, 256], f16, name="wfh_sb")
        ident = wpool.tile([P, P], f16, name="ident_sb")
        spair_sb = wpool.tile([P, meta["NPAIRC"]], f16, name="spair_sb")
        scut_sb = wpool.tile([P, meta["NCUTC"]], f16, name="scut_sb")
        stop_sb = wpool.tile([P, meta["NTOPC"]], f16, name="stop_sb")
        # Critical-path loads early on the SP queue; bulk loads staged on the
        # Act/Pool queues so no large transfer sits in front of an urgent one
        # (the DMA engines drain transfers in issue order).
        nc.sync.dma_start(wx_sb[:].rearrange("p t n -> p (t n)"), wxk[:])
        nc.sync.dma_start(ident[:], identd[:])
        nc.scalar.dma_start(wfh_sb[:].rearrange("p t n -> p (t n)"), wfhk[:])
        nc.scalar.dma_start(wh_sb[:].rearrange("p t n -> p (t n)"), whk[:])
        nc.scalar.dma_start(scut_sb[:], scutd[:])
        nc.gpsimd.dma_start(spair_sb[:, 0:meta["spA"]], spaird[:, 0:meta["spA"]])

        outT = wpool.tile([P, 2, NPHA], f16, name="outT_sb")
        contrib = [cpool.tile([P, 512], f16, name=f"contrib{t}")
                   for t in range(NBP)]
        topC0 = wpool.tile([P, 512], f16, name="topC0")
        xt_top = wpool.tile([P, NCH, TOPS], f16, name="xt_top")
        hT_top = wpool.tile([P, 2, P], f16, name="hT_top")
        fcT_top = wpool.tile([P, 2, P], f16, name="fcT_top")
        ccT_sb = wpool.tile([P, 4, CCW], f16, name="ccT_sb")
        # warm the activation table early (overlaps prologue DMAs)
        warm = wpool.tile([P, 2], f32, name="warm")
        nc.gpsimd.memset(warm[:], 0.0)
        nc.scalar.activation(warm[:, 0:1], warm[:, 1:2], SIG)
        nc.scalar.activation(warm[:, 0:1], warm[:, 1:2], TANH)
        # zero only the padded rows pair-matmuls will read (NaN hygiene)
        for t in range(NBP):
            cnt = pieces[t]["cnt"]
            if cnt < P:
                nc.gpsimd.memset(contrib[t][cnt:P, :], 0.0)
        nc.gpsimd.memset(topC0[:], 0.0)
        nc.gpsimd.memset(hT_top[:], 0.0)
        nc.gpsimd.memset(fcT_top[:], 0.0)

        ccps_h = pp_cc.tile([P, 2, CCW], f32, space="PSUM", name="ccps_h")
        ccps_f = pp_cc.tile([P, 2, CCW], f32, space="PSUM", name="ccps_f")

        pre_t = {}
        seg_t = {}
        emb_t = {}

        def prefetch_embs(T, s0=None, cnt=None):
            if s0 is None:
                pc = pieces[T]
                s0, cnt = pc["s0"], pc["cnt"]
            e4 = epool.tile([P, 4, P], f16, tag="e4", name=f"e4_{T}")
            e2 = epool.tile([K2, 2, P], f16, tag="e2", name=f"e2_{T}")
            nc.sync.dma_start(e4[:, :, :cnt], embs4[:, :, s0:s0 + cnt])
            nc.sync.dma_start(e2[:, :, :cnt], embs2[:, :, s0:s0 + cnt])
            emb_t[T] = (e4, e2)

        def emit_phase_a(pre, et, cnt, close_all, close_iou):
            # PSUM start zeroes the whole 2KB bank: assert start only on the
            # first matmul touching each bank (chunks 0-3 / 4-7)
            e4, e2 = et
            for ch in range(NCH):
                side = 0 if ch < 6 else 2
                w2 = 0 if ch < 6 else 1
                stop_ch = close_all or (ch < 6 and close_iou)
                for t in range(2):
                    nc.tensor.matmul(
                        pre[:, ch, :cnt],
                        lhsT=wx_sb[:, t, ch * P:(ch + 1) * P],
                        rhs=e4[:, side + t, :cnt],
                        start=(t == 0 and ch % 4 == 0), stop=False,
                        skip_group_check=True)
                nc.tensor.matmul(
                    pre[:, ch, :cnt],
                    lhsT=wx_sb[0:K2, 2, ch * P:(ch + 1) * P],
                    rhs=e2[:, w2, :cnt],
                    start=False, stop=stop_ch,
                    skip_group_check=True)

        def alloc_pre_bot(T):
            pc = pieces[T]
            pre = pp_pre.tile([P, NCH, P], f32, space="PSUM", tag="pre",
                              name=f"pre{T}")
            pre_t[T] = pre
            emit_phase_a(pre, emb_t.pop(T), pc["cnt"],
                         close_all=False, close_iou=(pc["level"] == 0))

        def emit_seg_into(T):
            """Segment-sum accumulation for piece T (emitted after T-1's chain)."""
            pc = pieces[T]
            cnt = pc["cnt"]
            seg = pp_seg.tile([P, 4, P], f32, space="PSUM", tag="seg",
                              name=f"seg{T}")
            seg_t[T] = seg
            if pc["kind"] in ("top", "root"):
                ti = T - NBP
                so = ti * P
                for ch in range(4):
                    nc.tensor.matmul(
                        seg[:, ch, :cnt], lhsT=topC0[:, ch * P:(ch + 1) * P],
                        rhs=stop_sb[:, so:so + cnt],
                        start=True, stop=False, skip_group_check=True)
                base = pc["base"]
                for ch in range(4):
                    nc.tensor.matmul(
                        seg[:, ch, :cnt], lhsT=ident[:],
                        rhs=ccT_sb[:, ch, base:base + cnt],
                        start=False, stop=True, skip_group_check=True)
            else:
                plist = pairs_union[T]
                for i, sp in enumerate(plist):
                    so = pair_off[(T, sp)]
                    for ch in range(4):
                        nc.tensor.matmul(
                            seg[:, ch, :cnt],
                            lhsT=contrib[sp][:, ch * P:(ch + 1) * P],
                            rhs=spair_sb[:, so:so + cnt],
                            start=(i == 0 and ch == 0),
                            stop=(i == len(plist) - 1),
                            skip_group_check=True)

        def emit_cut_pairs(T):
            if T not in cut_off:
                return
            so = cut_off[T]
            nonlocal cut_started
            for ch in range(4):
                ccp = ccps_h if ch < 2 else ccps_f
                nc.tensor.matmul(
                    ccp[:, ch % 2, :], lhsT=contrib[T][:, ch * P:(ch + 1) * P],
                    rhs=scut_sb[:, so:so + CCW],
                    start=(not cut_started and ch % 2 == 0),
                    stop=(T == cut_union[-1]),
                    skip_group_check=True)
            cut_started = True

        def emit_chain(T):
            pc = pieces[T]
            cnt = pc["cnt"]
            pre = pre_t[T]
            kind = pc["kind"]
            leaf = kind == "bot" and pc["level"] == 0
            top = kind in ("top", "root")
            base = pc.get("base", 0)
            col0 = (NBOT + base) if top else pc["s0"]

            if not leaf:
                seg = seg_t[T]
                hs = gpool.tile([P, 2, P], f16, tag="hs", name=f"hs{T}")
                nc.scalar.activation(hs[:, :, :cnt], seg[:, 0:2, :cnt], COPY)
                fcs = gpool.tile([P, 2, P], f16, tag="fcs", name=f"fcs{T}")
                nc.vector.tensor_copy(fcs[:, :, :cnt], seg[:, 2:4, :cnt])
                if top:
                    for ch in range(NCH):
                        nc.tensor.matmul(
                            pre[:, ch, :cnt], lhsT=ident[:],
                            rhs=xt_top[:, ch, base:base + cnt],
                            start=True, stop=False, skip_group_check=True)
                for ch in range(6):
                    for t in range(2):
                        nc.tensor.matmul(
                            pre[:, ch, :cnt],
                            lhsT=wh_sb[:, t, ch * P:(ch + 1) * P],
                            rhs=hs[:, t, :cnt],
                            start=False, stop=(t == 1), skip_group_check=True)

            io = gpool.tile([P, 4, P], f16, tag="io", name=f"io{T}")
            u = gpool.tile([P, 2, P], f16, tag="u", name=f"u{T}")
            nc.scalar.activation(io[:, :, :cnt], pre[:, 0:4, :cnt], SIG)
            nc.scalar.activation(u[:, :, :cnt], pre[:, 4:6, :cnt], TANH)
            c_sb = gpool.tile([P, 2, P], f16, tag="c", name=f"c{T}")
            nc.vector.tensor_mul(c_sb[:, :, :cnt], io[:, 0:2, :cnt],
                                 u[:, :, :cnt])
            if not leaf:
                nc.vector.tensor_add(c_sb[:, :, :cnt], c_sb[:, :, :cnt],
                                     fcs[:, :, :cnt])
            tc_sb = gpool.tile([P, 2, P], f16, tag="tc", name=f"tc{T}")
            nc.scalar.activation(tc_sb[:, :, :cnt], c_sb[:, :, :cnt], TANH)
            if kind == "top":
                hdst = hT_top[:, :, base:base + cnt]
            else:
                hdst = outT[:, :, col0:col0 + cnt]
            nc.vector.tensor_mul(hdst, io[:, 2:4, :cnt], tc_sb[:, :, :cnt])
            if kind == "root":
                return
            for ch in range(2):
                for t in range(2):
                    nc.tensor.matmul(
                        pre[:, 6 + ch, :cnt],
                        lhsT=wfh_sb[:, t, ch * P:(ch + 1) * P],
                        rhs=hdst[:, t, :],
                        start=False, stop=(t == 1), skip_group_check=True)
            f_sb = gpool.tile([P, 2, P], f16, tag="f", name=f"f{T}")
            nc.scalar.activation(f_sb[:, :, :cnt], pre[:, 6:8, :cnt], SIG)
            if kind == "top":
                fcdst = fcT_top[:, :, base:base + cnt]
            else:
                fc = gpool.tile([P, 2, P], f16, tag="fc", name=f"fc{T}")
                fcdst = fc[:, :, :cnt]
            nc.vector.tensor_mul(fcdst, f_sb[:, :, :cnt], c_sb[:, :, :cnt])

            if kind == "top":
                srcs = [hT_top[:, 0, :], hT_top[:, 1, :],
                        fcT_top[:, 0, :], fcT_top[:, 1, :]]
                w = P
                dst = topC0
            else:
                srcs = [hdst[:, 0, :], hdst[:, 1, :],
                        fcdst[:, 0, :], fcdst[:, 1, :]]
                w = cnt
                dst = contrib[T]
            # transpose into a contiguous f16 strip of the (consumed) seg
            # tile, then evacuate all 512 contrib columns with ONE copy
            if leaf:
                scr = pp_seg.tile([P, 4, P], f32, space="PSUM", tag="seg",
                                  name=f"scr{T}")
            else:
                scr = seg
            sv = scr[:, :, :].bitcast(f16)  # [P, 4, 256]
            for k in range(4):
                nc.tensor.transpose(sv[0:w, k // 2, (k % 2) * P:(k % 2 + 1) * P],
                                    in_=srcs[k], identity=ident[:])
            nc.vector.tensor_copy(
                dst[0:w, :],
                sv[0:w, 0:2, :].rearrange("p a n -> p (a n)"))

        # ---------------- program ----------------
        cut_started = False
        prefetch_embs(0)
        prefetch_embs(1)
        prefetch_embs(2)
        alloc_pre_bot(0)
        alloc_pre_bot(1)

        for T in range(NTOT):
            pc = pieces[T]
            if T == NBP:
                # cut AllReduce: h and fc halves pipelined independently
                cc_sb = gpool.tile([P, 4, CCW], f16, tag="ccsb", name="cc_sb")
                nc.vector.tensor_copy(cc_sb[:, 0:2, :], ccps_h[:, :, :])
                nc.scalar.activation(cc_sb[:, 2:4, :], ccps_f[:, :, :], COPY)
                halves = [(cc_in_h, cc_out_h, 0), (cc_in_f, cc_out_f, 2)]
                for (ci, co, k0) in halves:
                    nc.sync.dma_start(
                        ci[:], cc_sb[:, k0:k0 + 2, :].rearrange(
                            "p a n -> p (a n)"))
                if sim_no_collective:
                    for (ci, co, k0) in halves:
                        cpy = gpool.tile([P, 2 * CCW], f16, tag=f"cpy{k0}",
                                         name=f"cc_cpy{k0}")
                        nc.sync.dma_start(cpy[:], ci[:])
                        nc.sync.dma_start(co[:], cpy[:])
                else:
                    for (ci, co, k0) in halves:
                        nc.gpsimd.collective_compute(
                            "AllReduce", mybir.AluOpType.add,
                            replica_groups=[list(range(N_CORES))],
                            ins=[ci[:]], outs=[co[:]],
                        )
                for (ci, co, k0) in halves:
                    nc.sync.dma_start(
                        ccT_sb[:, k0:k0 + 2, :].rearrange("p a n -> p (a n)"),
                        co[:])

            if pc["kind"] != "bot":
                pre_t[T] = pp_pre.tile([P, NCH, P], f32, space="PSUM",
                                       tag="pre", name=f"pre{T}")
            emit_chain(T)
            nxt = T + 1
            if nxt < NTOT and not (pieces[nxt]["kind"] == "bot"
                                   and pieces[nxt]["level"] == 0):
                emit_seg_into(nxt)
            if pc["kind"] == "bot":
                emit_cut_pairs(T)
            if T == 2:
                nc.gpsimd.dma_start(spair_sb[:, meta["spA"]:meta["spB"]],
                                    spaird[:, meta["spA"]:meta["spB"]])
            if T == 6:
                nc.gpsimd.dma_start(spair_sb[:, meta["spB"]:],
                                    spaird[:, meta["spB"]:])
                nc.gpsimd.dma_start(stop_sb[:], stopd[:])
            if T == NBP - 1:
                nc.sync.dma_start(outT_d[:, :, 0:NBOT], outT[:, :, 0:NBOT])
            if T + 3 < NBP:
                prefetch_embs(T + 3)
            if T + 2 < NBP:
                alloc_pre_bot(T + 2)
            if T == NBP - 3:
                # top phase A -> xt_top, emitted early so the AllReduce
                # window only carries the collective itself
                for (b0, bw) in ((0, P), (P, TOPS - P)):
                    ptop = pp_pre.tile([P, NCH, P], f32, space="PSUM",
                                       tag="pre", name=f"pretop{b0}")
                    prefetch_embs(1000 + b0, s0=NBOT + b0, cnt=bw)
                    emit_phase_a(ptop, emb_t.pop(1000 + b0), bw,
                                 close_all=True, close_iou=True)
                    nc.vector.tensor_copy(xt_top[:, 0:4, b0:b0 + bw],
                                          ptop[:, 0:4, :bw])
                    nc.scalar.activation(xt_top[:, 4:8, b0:b0 + bw],
                                         ptop[:, 4:8, :bw], COPY)

        rc = NBOT + P  # root's phase-A column
        nc.sync.dma_start(outT_d[:, :, rc:rc + 1], outT[:, :, rc:rc + 1])
        nc.sync.dma_start(topC_d[:], topC0[:])

    nc.compile()
    return nc


# ---------------------------------------------------------------- entry point

_CACHE = {}


def _get_program(parent_bytes):
    if parent_bytes not in _CACHE:
        parent = np.frombuffer(parent_bytes, dtype=np.int64)
        meta = _preprocess(parent)
        nc = _build_program(meta)
        _CACHE[parent_bytes] = (meta, nc)
    return _CACHE[parent_bytes]


def kernel(embs, parent, Wx, bx, Wh, bh, Wfh, bfh):
    from concourse.bass_utils import run_bass_kernel_spmd

    embs = np.asarray(embs, np.float32)
    parent = np.asarray(parent, np.int64)
    meta, nc = _get_program(parent.tobytes())
    in_maps = _build_inputs(
        meta, embs,
        np.asarray(Wx, np.float32), np.asarray(bx, np.float32),
        np.asarray(Wh, np.float32), np.asarray(bh, np.float32),
        np.asarray(Wfh, np.float32), np.asarray(bfh, np.float32))
    res = run_bass_kernel_spmd(nc, in_maps, list(range(N_CORES)))
    return _assemble(meta, res.results)


def _assemble(meta, outs):
    N = meta["N"]
    NBOT = meta["NBOT"]
    h = np.zeros((N, MD), dtype=np.float32)
    for c in range(N_CORES):
        na = meta["node_at"][c]
        oT = np.asarray(outs[c]["outT"], dtype=np.float32).reshape(P, 2, -1)
        m = na[:NBOT] >= 0
        sl = np.arange(NBOT)[m]
        h[na[sl], 0:P] = oT[:, 0, sl].T
        h[na[sl], P:2 * P] = oT[:, 1, sl].T
    hT = np.asarray(outs[0]["htop"], dtype=np.float32).reshape(P, 2, P)
    na0 = meta["node_at"][0]
    for ts in range(P):
        node = na0[NBOT + ts]
        if node >= 0:
            h[node, 0:P] = hT[:, 0, ts]
            h[node, P:2 * P] = hT[:, 1, ts]
    oT0 = np.asarray(outs[0]["outT"], dtype=np.float32).reshape(P, 2, -1)
    h[N - 1, 0:P] = oT0[:, 0, NBOT + P]
    h[N - 1, P:2 * P] = oT0[:, 1, NBOT + P]
    return h
